# revision 16
# baseline (speedup 1.0000x reference)
"""MultiHeadAttention forward on 8 Trainium2 NeuronCores.

Tensor-parallel over heads: each core owns 2 of 16 heads (d_loc=256 of the
2048 QKV output columns, and the matching 256 rows of Wo). Each core
computes a full-shape partial output in bf16; the host sums the 8 partials
and adds bo (+ bv @ Wo, folding the V bias through the row-stochastic
attention weights).

Problem shape: x [2, 2048, 2048], 16 heads, d_k = 128.

Device-side layout (all matmul operands bf16, PSUM accumulation fp32):
  - x is fed pre-transposed (xT [C, B*T] bf16) and kept SBUF-resident one
    half-batch ([128, 1024] tiles) at a time.
  - Q, K are produced transposed (QT/KT [d, t] bf16); V natural [t, d].
  - Scores are computed transposed: ST[tk, tq] = matmul(lhsT=KT-tile,
    rhs=QT-chunk); softmax needs no max-subtraction (|scores| ~ 5).
  - exp on ScalarE (PSUM -> bf16 SBUF, pairs of tk tiles share one
    [128, 1024] es buffer); AV as matmul(lhsT=V-tile, rhs=es half)
    accumulating out^T [d, tq] over all tk. The AV matmuls are emitted
    TWO tk steps behind the scores matmuls so they never sit at the PE
    queue head waiting for ScalarE (which paces the attention loop).
  - Softmax denominator: DVE accumulates the 8 es pair-buffers
    elementwise (bf16, 2x mode) into es_sum [128, 1024]; two accumulating
    ones-matmuls reduce over the partition dim -> pdn [128, 512]. This
    replaces 16 ones-matmuls per chunk with 2 (PE 917k -> 819k cycles).
  - Normalization (1/denom) on DVE applied to out^T chunks -> avT bf16.
  - Output projection from avT slices; PSUM -> bf16 SBUF staging copies on
    DVE during attention phases and alternating DVE/ScalarE in drain
    phases; y written bf16.
  - Cross-phase software pipelining: batch b+1's QKV projection and batch
    b's output projection are emitted in small quanta interleaved into the
    attention instruction stream, filling PE idle gaps.
"""

import functools
from contextlib import ExitStack

import numpy as np
import ml_dtypes

D_MODEL = 2048
NUM_HEADS = 16
DK = 128
B = 2
T = 2048
BT = B * T
N_CORES = 8
H_LOC = NUM_HEADS // N_CORES  # 2 heads per core
D_LOC = H_LOC * DK  # 256
C_TILES = D_MODEL // 128  # 16
TQ = 512  # tq chunk width (one PSUM bank in fp32)
NCH = T // TQ  # 4 chunks per batch
TK_TILES = T // 128  # 16
NPAIR = TK_TILES // 2  # 8 tk-tile pairs per chunk
AV_LAG = 2  # tk steps the AV matmuls trail the scores matmuls by
HB = 1024  # tokens per resident x half-batch
NXG = BT // HB  # 4 x half-batch groups

BF16 = np.dtype(ml_dtypes.bfloat16)


class Feeder:
    """Queue of generators; each next() emits one small quantum (~0.9 us of
    PE work) of deferred projection / output-projection instructions."""

    def __init__(self):
        self.gens = []

    def add(self, gen):
        self.gens.append(gen)

    def emit(self, k):
        while k > 0 and self.gens:
            try:
                next(self.gens[0])
                k -= 1
            except StopIteration:
                self.gens.pop(0)

    def drain(self):
        while self.gens:
            self.emit(1 << 30)


def _body(ctx, tc, xT, wqkv, bqk, wo, y):
    import concourse.bass as bass  # noqa: F401
    from concourse import mybir

    nc = tc.nc
    f32 = mybir.dt.float32
    bf16 = mybir.dt.bfloat16
    Exp = mybir.ActivationFunctionType.Exp
    inv_sqrt_dk = 1.0 / float(np.sqrt(DK))

    # ---------------- pools ----------------
    wpool = ctx.enter_context(tc.tile_pool(name="wpool", bufs=1))
    x_pool = ctx.enter_context(tc.tile_pool(name="x_pool", bufs=1))
    qkv_pool = ctx.enter_context(tc.tile_pool(name="qkv_pool", bufs=2))
    av_pool = ctx.enter_context(tc.tile_pool(name="av_pool", bufs=2))
    es_pool = ctx.enter_context(tc.tile_pool(name="es_pool", bufs=4))
    sum_pool = ctx.enter_context(tc.tile_pool(name="sum_pool", bufs=2))
    rc_pool = ctx.enter_context(tc.tile_pool(name="rc_pool", bufs=2))
    y_pool = ctx.enter_context(tc.tile_pool(name="y_pool", bufs=3))

    # PSUM: 8 banks of [128, 512] fp32.
    ps_s = ctx.enter_context(tc.tile_pool(name="ps_s", bufs=2, space="PSUM"))
    ps_av = ctx.enter_context(tc.tile_pool(name="ps_av", bufs=1, space="PSUM"))
    ps_dn = ctx.enter_context(tc.tile_pool(name="ps_dn", bufs=1, space="PSUM"))
    ps_proj = ctx.enter_context(tc.tile_pool(name="ps_proj", bufs=2, space="PSUM"))
    ps_o = ctx.enter_context(tc.tile_pool(name="ps_o", bufs=2, space="PSUM"))

    # ---------------- resident tensors / startup DMAs ----------------
    w_tiles = []
    xg_tiles = [[None] * C_TILES for _ in range(NXG)]

    def load_xg(g):
        for i in range(C_TILES):
            xt = x_pool.tile([128, HB], bf16, tag=f"x{g % 2}_{i}", name=f"x{g}_{i}")
            nc.sync.dma_start(
                out=xt, in_=xT[i * 128 : (i + 1) * 128, g * HB : (g + 1) * HB]
            )
            xg_tiles[g][i] = xt

    bqk_sb = wpool.tile([128, 4], f32, tag="bqk", name="bqk")
    nc.sync.dma_start(out=bqk_sb, in_=bqk[:, :])
    for i in range(C_TILES):
        xt = x_pool.tile([128, HB], bf16, tag=f"x0_{i}", name=f"x0_{i}")
        nc.sync.dma_start(out=xt, in_=xT[i * 128 : (i + 1) * 128, 0:HB])
        xg_tiles[0][i] = xt
        wt = wpool.tile([128, 3 * D_LOC], bf16, tag=f"w{i}", name=f"w{i}")
        nc.sync.dma_start(out=wt, in_=wqkv[i * 128 : (i + 1) * 128, :])
        w_tiles.append(wt)

    ones = wpool.tile([128, 128], bf16, tag="ones", name="ones")
    nc.vector.memset(ones, 1.0)
    # Touch Exp early so the ACT table load happens during the projection
    # phase instead of at the first attention chunk.
    warm = wpool.tile([128, 8], f32, tag="warm", name="warm")
    nc.scalar.activation(warm, ones[:, 0:8], Exp)

    load_xg(1)
    wo_tiles = []
    for d in range(2):
        wot = wpool.tile([128, D_MODEL], bf16, tag=f"wo{d}", name=f"wo{d}")
        nc.sync.dma_start(out=wot, in_=wo[d * 128 : (d + 1) * 128, :])
        wo_tiles.append(wot)

    # ---------------- phase generators ----------------
    def proj_gen(b, store):
        """QKV projection for batch b, yielded in ~4-matmul quanta."""
        qT = [
            qkv_pool.tile([128, T], bf16, tag=f"qT{d}", name=f"qT{d}_{b}")
            for d in range(2)
        ]
        kT = [
            qkv_pool.tile([128, T], bf16, tag=f"kT{d}", name=f"kT{d}_{b}")
            for d in range(2)
        ]
        v_t = [
            qkv_pool.tile([128, D_LOC], bf16, tag=f"v{t}", name=f"v{t}_{b}")
            for t in range(TK_TILES)
        ]
        store["qT"], store["kT"], store["v"] = qT, kT, v_t

        for ch in range(NCH):
            g = 2 * b + ch // 2
            xoff = (ch % 2) * TQ
            xg = xg_tiles[g]
            # QT / KT: j -> (qT, kT)[j // 2][j % 2]
            for j, dest in enumerate((qT[0], qT[1], kT[0], kT[1])):
                ps = ps_proj.tile([128, TQ], f32, tag="proj", name=f"psqk{b}_{ch}_{j}")
                for i in range(C_TILES):
                    nc.tensor.matmul(
                        ps,
                        w_tiles[i][:, j * 128 : (j + 1) * 128],
                        xg[i][:, xoff : xoff + TQ],
                        start=(i == 0),
                        stop=(i == C_TILES - 1),
                    )
                    if i % 4 == 3 and i < C_TILES - 1:
                        yield
                nc.vector.tensor_scalar_add(
                    dest[:, ch * TQ : (ch + 1) * TQ], ps, bqk_sb[:, j : j + 1]
                )
                yield
            # V: natural [t, d] layout
            for ts in range(TQ // 128):
                t_idx = ch * (TQ // 128) + ts
                ps = ps_proj.tile([128, TQ], f32, tag="proj", name=f"psv{b}_{t_idx}")
                psv = ps[:, :D_LOC]
                for i in range(C_TILES):
                    nc.tensor.matmul(
                        psv,
                        xg[i][:, xoff + ts * 128 : xoff + (ts + 1) * 128],
                        w_tiles[i][:, 2 * D_LOC : 3 * D_LOC],
                        start=(i == 0),
                        stop=(i == C_TILES - 1),
                    )
                    if i % 8 == 7 and i < C_TILES - 1:
                        yield
                nc.vector.tensor_copy(v_t[t_idx], psv)
                yield

    # Engine used for the O-projection PSUM -> SBUF staging copies: DVE
    # during attention phases (ScalarE is saturated by exp there),
    # alternating DVE/ScalarE during the drain phases. GpSimd cannot
    # access PSUM on TRN2.
    copy_eng = {"eng": "vector", "n": 0}

    def stage_copy(ys, pso):
        eng = copy_eng["eng"]
        if eng == "alt":
            copy_eng["n"] += 1
            eng = "vector" if copy_eng["n"] % 2 else "scalar"
        if eng == "vector":
            nc.vector.tensor_copy(ys, pso)
        else:
            nc.scalar.copy(ys, pso)

    def o_gen(b, ch, avT):
        """Output projection for the 4 t-tiles of chunk ch of batch b."""
        for t in range(ch * (TQ // 128), (ch + 1) * (TQ // 128)):
            row0 = b * T + t * 128
            for q4 in range(4):
                pso = ps_o.tile([128, TQ], f32, tag="o", name=f"pso{b}_{t}_{q4}")
                for d in range(2):
                    nc.tensor.matmul(
                        pso,
                        avT[d][:, t * 128 : (t + 1) * 128],
                        wo_tiles[d][:, q4 * TQ : (q4 + 1) * TQ],
                        start=(d == 0),
                        stop=(d == 1),
                    )
                ys = y_pool.tile([128, TQ], bf16, tag="ys", name=f"ys{b}_{t}_{q4}")
                stage_copy(ys, pso)
                nc.sync.dma_start(
                    out=y[row0 : row0 + 128, q4 * TQ : (q4 + 1) * TQ], in_=ys
                )
                yield

    def attn_chunk(b, h, ch, qT, kT, v_t, avT, feeder):
        # Deferred quanta first: they execute while this chunk's first
        # scores matmul is still waiting on the previous chunk's avT
        # normalization to release the AV accumulator bank.
        feeder.emit(2)
        pav = ps_av.tile([128, TQ], f32, tag="av", name=f"pav{b}_{h}_{ch}")
        es_tiles = []
        # es_sum is split: GpSimd (otherwise idle, SBUF-only engine)
        # accumulates pairs 0-2 while DVE accumulates pairs 3-7; DVE merges
        # at the end. This keeps DVE off the critical path in the attention
        # phases where it would otherwise co-saturate with ScalarE.
        essA, essB = [None], [None]

        def scores_step(tk):
            p, s = divmod(tk, 2)
            if s == 0:
                es_tiles.append(
                    es_pool.tile(
                        [128, 2 * TQ], bf16, tag="es", name=f"es{b}_{h}_{ch}_{p}"
                    )
                )
            es = es_tiles[p]
            pss = ps_s.tile([128, TQ], f32, tag="s", name=f"pss{b}_{h}_{ch}_{tk}")
            nc.tensor.matmul(
                pss,
                kT[h][:, tk * 128 : (tk + 1) * 128],
                qT[h][:, ch * TQ : (ch + 1) * TQ],
                start=True,
                stop=True,
            )
            nc.scalar.activation(es[:, s * TQ : (s + 1) * TQ], pss, Exp, scale=inv_sqrt_dk)
            if s == 1:
                # pair complete -> accumulate into an es_sum accumulator
                if p < 3:
                    if essB[0] is None:
                        essB[0] = sum_pool.tile(
                            [128, 2 * TQ], bf16, tag="essB", name=f"essB{b}_{h}_{ch}"
                        )
                        nc.gpsimd.tensor_copy(essB[0], es)
                    else:
                        nc.gpsimd.tensor_add(essB[0], essB[0], es)
                else:
                    if essA[0] is None:
                        essA[0] = sum_pool.tile(
                            [128, 2 * TQ], bf16, tag="essA", name=f"essA{b}_{h}_{ch}"
                        )
                        nc.vector.tensor_copy(essA[0], es)
                    else:
                        nc.vector.tensor_add(essA[0], essA[0], es)

        def av_step(tk):
            p, s = divmod(tk, 2)
            nc.tensor.matmul(
                pav,
                v_t[tk][:, h * 128 : (h + 1) * 128],
                es_tiles[p][:, s * TQ : (s + 1) * TQ],
                start=(tk == 0),
                stop=(tk == TK_TILES - 1),
            )

        for tk in range(TK_TILES):
            scores_step(tk)
            if tk >= AV_LAG:
                av_step(tk - AV_LAG)
            if tk in (5, 9, 13):
                feeder.emit(1)
        # Trailing AV matmuls: space them with deferred quanta so they are
        # not parked at the PE queue head waiting for the last exps.
        av_step(TK_TILES - 2)
        feeder.emit(1)
        av_step(TK_TILES - 1)
        nc.vector.tensor_add(essA[0], essA[0], essB[0])
        feeder.emit(1)
        pdn = ps_dn.tile([128, TQ], f32, tag="dn", name=f"pdn{b}_{h}_{ch}")
        nc.tensor.matmul(pdn, ones, essA[0][:, 0:TQ], start=True, stop=False)
        nc.tensor.matmul(pdn, ones, essA[0][:, TQ : 2 * TQ], start=False, stop=True)
        rc = rc_pool.tile([128, TQ], f32, tag="rc", name=f"rc{b}_{h}_{ch}")
        nc.vector.reciprocal_approx_fast(out=rc, in_=pdn)
        nc.vector.tensor_mul(avT[h][:, ch * TQ : (ch + 1) * TQ], pav, rc)

    # ---------------- schedule ----------------
    stores = [{}, {}]
    f_p0 = Feeder()
    f_p0.add(proj_gen(0, stores[0]))
    f_p0.drain()

    load_xg(2)

    feeder = Feeder()
    feeder.add(proj_gen(1, stores[1]))

    for b in range(B):
        qT, kT, v_t = stores[b]["qT"], stores[b]["kT"], stores[b]["v"]
        avT = [
            av_pool.tile([128, T], bf16, tag=f"avT{d}", name=f"avT{d}_{b}")
            for d in range(2)
        ]
        for ch in range(NCH):
            for h in range(H_LOC):
                attn_chunk(b, h, ch, qT, kT, v_t, avT, feeder)
            feeder.add(o_gen(b, ch, avT))
            if b == 0 and ch == 1:
                load_xg(3)
        if b == 0:
            # Finish batch 1's projection and batch 0's output projection
            # before attention on batch 1 begins.
            copy_eng["eng"] = "alt"
            feeder.drain()
            copy_eng["eng"] = "vector"
            feeder = Feeder()
    copy_eng["eng"] = "alt"
    feeder.drain()


@functools.cache
def _build():
    from concourse import bacc
    import concourse.tile as tile
    from concourse import mybir

    nc = bacc.Bacc(
        "TRN2",
        target_bir_lowering=False,
        debug=False,
        enable_asserts=False,
        num_devices=N_CORES,
    )
    f32 = mybir.dt.float32
    bf16 = mybir.dt.bfloat16
    xT = nc.dram_tensor("xT", [D_MODEL, BT], bf16, kind="ExternalInput").ap()
    wqkv = nc.dram_tensor("wqkv", [D_MODEL, 3 * D_LOC], bf16, kind="ExternalInput").ap()
    bqk = nc.dram_tensor("bqk", [128, 4], f32, kind="ExternalInput").ap()
    wo = nc.dram_tensor("wo", [D_LOC, D_MODEL], bf16, kind="ExternalInput").ap()
    y = nc.dram_tensor("y", [BT, D_MODEL], bf16, kind="ExternalOutput").ap()

    with tile.TileContext(nc) as tc:
        with ExitStack() as ctx:
            _body(ctx, tc, xT, wqkv, bqk, wo, y)
    nc.compile()
    return nc


def _shard_inputs(x, Wq, bq, Wk, bk, Wv, bv, Wo, bo):
    """Host-side sharding: returns per-core input maps."""
    f = np.float32
    xT = np.ascontiguousarray(np.asarray(x, f).reshape(BT, D_MODEL).T.astype(BF16))
    Wq, Wk, Wv, Wo = (np.asarray(a, f) for a in (Wq, Wk, Wv, Wo))
    bq, bk, bv = (np.asarray(a, f) for a in (bq, bk, bv))
    in_maps = []
    for c in range(N_CORES):
        sl = slice(c * D_LOC, (c + 1) * D_LOC)
        wqkv_pad = np.ascontiguousarray(
            np.concatenate([Wq[:, sl], Wk[:, sl], Wv[:, sl]], axis=1).astype(BF16)
        )
        bqk_t = np.ascontiguousarray(
            np.stack(
                [
                    bq[sl][:128],
                    bq[sl][128:],
                    bk[sl][:128],
                    bk[sl][128:],
                ],
                axis=1,
            )
        )
        wo_loc = np.ascontiguousarray(Wo[sl, :].astype(BF16))
        in_maps.append({"xT": xT, "wqkv": wqkv_pad, "bqk": bqk_t, "wo": wo_loc})
    return in_maps


def _run(in_maps, trace=False, **kwargs):
    from concourse.bass_utils import run_bass_kernel_spmd

    nc = _build()
    return run_bass_kernel_spmd(
        nc, in_maps, core_ids=list(range(N_CORES)), trace=trace, **kwargs
    )


def kernel(x, Wq, bq, Wk, bk, Wv, bv, Wo, bo):
    in_maps = _shard_inputs(x, Wq, bq, Wk, bk, Wv, bv, Wo, bo)
    res = _run(in_maps, trace=False)
    acc = np.zeros((BT, D_MODEL), np.float32)
    for rmap in res.results:
        acc += np.asarray(rmap["y"], np.float32)
    acc += np.asarray(bo, np.float32)[None, :]
    acc += (np.asarray(bv, np.float32) @ np.asarray(Wo, np.float32))[None, :]
    return acc.reshape(B, T, D_MODEL)
